# revision 57
# baseline (speedup 1.0000x reference)
"""Causal self-attention with RoPE on 8 Trainium2 NeuronCores.

Sharding: Megatron-style head parallelism. 16 heads / 8 cores = 2 heads per
core. Each core computes q/k/v projections for its 2 heads (column-parallel),
full causal attention for those heads, and a partial output projection
(row-parallel slice of w_o). The host sums the 8 partial outputs.

On-chip layout: everything transposed. Host passes xT = x^T per batch
[B, D, T]; projections produce qT/kT [dh, t] directly (lhsT = weight slice,
rhs = xT chunk) and v [t, dh] (lhsT = xT chunk, rhs = w_v slice). Scores are
computed transposed, ST[kv, q] = matmul(lhsT=kT_chunk, rhs=qT_group), which
makes P^T directly usable as the moving operand of the PV matmul - no
on-chip transposes anywhere. The causal mask is accumulated onto the score
PSUM by an identity-matmul against an additive -1e30 mask slice (PE-side:
no cross-engine latency on the exp input).

All matmul operands are float32r (full PE rate at moving-dim >= 256, and -
unlike bf16 - the fp32r matmul self-loads its stationary, overlapping the
weight load with the stream; bf16 operands emit separate LDWEIGHTS
instructions that serialize ~100ns+ per dependent matmul, measured; the
walrus verifier also forbids mixing f32r with bf16 matmul operands).

Schedule (the changes that took the 952us baseline to ~800us):
- Merged phases: attention for q-group qi=tt runs right after projection
  tile tt (it only needs qT/kT/vv from tiles <= tt), so attention hides
  batch 0's DMA-bound start and the phase/batch boundary stalls.
- Column-restricted diagonal tiles: the kv tile at diagonal offset dg
  only attends queries >= dg*TK within its group, so score/exp/PV/sum
  all skip the fully-masked columns, and the additive causal mask
  shrinks to one TK x TK triangle applied by a 128-col identity matmul
  (measured ~-37us of PE busy vs full-width diagonals).
- Both heads interleave within each q-group, and PV + denominator-sum
  matmuls are emitted one kv-tile BEHIND the scores: the ACT exp latency
  (~650ns) hides under two kv-tiles (~1.4us) of PE work.
- Off-diagonal exp tiles are pre-summed in quads of 4 on the otherwise
  idle GpSimd engine; one ones-matmul per quad (flushed at group end,
  covered by the diagonal span) replaces four, quartering the PE cost
  of the softmax denominators for the off-diagonal bulk.
- The unnormalized attention output is copied PSUM->SBUF by one ACT op
  at group end, freeing the psO bank immediately (the 2-deep psO ring
  otherwise makes the next group's first PV wait out the ~3.5us
  normalization chain); the chain then scales that copy in place.
- The softmax reciprocal is exp(-ln(den)) on ACT [1,TQ] (Ln/Exp/Copy share
  one activation table set, so no ACT_TABLE_LOAD; the baseline's DVE
  reciprocal on [128,TQ] cost 3.3us x32 = 107us of DVE). It is deferred
  one kv-tile into the next group so the LN/exp never sit between
  consecutive exps in the in-order ACT queue; a ones-matmul broadcasts
  the reciprocal across partitions into a psY-ring bank.
- RoPE reads a fast ACT copy of the projection PSUM (if the DVE reads the
  PSUM directly, the 2-bank st-ring couples the PE to DVE backlog:
  measured ~3.9us stalls plus p-state resets); the rotate-half partition
  swap runs on the DMA engine (the DVE cannot pair SBUF operands at
  different start partitions); the multiply/add run in bf16.
- The out-projection is emitted in 4 q-chunks popped between kv tiles of
  LATER groups (a 16-copy ysb burst ahead of the next exps otherwise
  stalls the PE via the in-order DVE/ACT queues); output y is written in
  bf16 (halves output DMA); the host upcasts and sums the 8 partials.
- TT=512 projections (half the matmul instruction count of TT=256); x
  tiles are split into two half-depth chunks to fit SBUF, with a one-slot
  DMA prefetch ahead of the attention's y-output traffic; wo loads are
  deferred behind batch 0's x tiles; q lives in a small per-tile ring
  (only its own q-group ever reads it).

Measured dead ends (don't revisit): walrus rejects matmul PSUM dst
partition offsets (s3d3_mm_valid_dst_partition), so col-tiled
concurrent M=1 sum matmuls at partitions 32/64/96 won't compile;
x-in-bf16 with on-chip upcast loses ~130us (GpSimd/ACT copies stall
the projections; GpSimd tensor_copy of a [128,4,512] block measures
~7us); splitting startup DMA chunks in half doubles descriptor count
and makes the DMA-queue-limited startup WORSE.

The attention scale 1/sqrt(dh) is folded into w_q on the host. No
max-subtraction: logits are q.k/sqrt(dh) with unit-ish variance,
|logit| << 88, identical math to the reference.
"""

import numpy as np

B, T, D = 4, 2048, 2048
H, DH = 16, 128
NCORES = 8
HPC = H // NCORES  # heads per core
THETA = 10000.0

TT = 512  # projection t-tile (moving dim of q/k projection matmuls)
TQ = 512  # attention q-group width
TK = 128  # kv tile (contraction chunk of PV / partition dim of ST)


def _rope_tables(seq_len, d_head, theta):
    # Matches reference.rope_cos_sin numerics, then transposes to [dh, t]
    # and folds the rotate-half sign into sin.
    inv_freq = 1.0 / (theta ** (np.arange(0, d_head, 2, dtype=np.float32) / d_head))
    t = np.arange(seq_len, dtype=np.float32)
    freqs = np.einsum("i,j->ij", t, inv_freq)
    emb = np.concatenate([freqs, freqs], axis=-1)  # [T, dh]
    cosT = np.ascontiguousarray(np.cos(emb).astype(np.float32).T)  # [dh, T]
    sinT = np.ascontiguousarray(np.sin(emb).astype(np.float32).T)
    sgn = np.ones((d_head, 1), np.float32)
    sgn[: d_head // 2] = -1.0
    return cosT, sinT * sgn


def _causal_mask_add(tk):
    # Additive causal triangle [tk, tk]: with column-restricted diagonal
    # tiles the only region that ever needs masking is the tk x tk block
    # on the diagonal itself: entry is -1e30 (masked) iff c < r.
    m = np.zeros((tk, tk), np.float32)
    for r in range(tk):
        m[r, :r] = -1e30
    return m


def _legalize_waits(nc, mybir):
    """Walrus on this toolchain refuses more than one embedded sync wait
    per engine instruction. Hoist extra waits into standalone
    EventSemaphore instructions on the same engine queue (the sequencer
    executes them in-stream before the instruction, same gating)."""
    n = 0
    for f in nc.m.functions:
        for bb in f.blocks:
            out = []
            for inst in bb.instructions:
                si = inst.sync_info
                if (si and si.on_wait and len(si.on_wait) > 1
                        and not isinstance(inst, mybir.InstEventSemaphore)):
                    for w in si.on_wait[:-1]:
                        out.append(mybir.InstEventSemaphore(
                            name=f"WH-{n}", engine=inst.engine,
                            sync_info=mybir.SyncInfo(
                                on_wait=[w], on_update=[])))
                        n += 1
                    inst.sync_info = mybir.SyncInfo(
                        on_wait=[si.on_wait[-1]],
                        on_update=list(si.on_update))
                out.append(inst)
            bb.instructions = out
    return n


def _build_nc(b_sz, t_sz, d_sz, legalize=True):
    import concourse.bass as bass
    import concourse.tile as tile
    from concourse import mybir

    f32 = mybir.dt.float32
    f32r = mybir.dt.float32r
    bf16 = mybir.dt.bfloat16
    EXP = mybir.ActivationFunctionType.Exp
    LN = mybir.ActivationFunctionType.Ln

    DC = d_sz // 128         # contraction chunks
    DCH = DC // 2            # chunks per x half-tile
    NQG = t_sz // TQ         # q groups per (batch, head)
    NKT = t_sz // TK         # kv tiles
    KPG = TQ // TK           # kv tiles per q group (diagonal span)

    nc = bass.Bass("TRN2", target_bir_lowering=False, debug=False,
                   enable_asserts=False, dynamic_dma_scratch_size=2048)

    xT = nc.dram_tensor("xT", [b_sz, d_sz, t_sz], f32, kind="ExternalInput")
    wq = nc.dram_tensor("wq", [d_sz, HPC * DH], f32, kind="ExternalInput")
    wk = nc.dram_tensor("wk", [d_sz, HPC * DH], f32, kind="ExternalInput")
    wv = nc.dram_tensor("wv", [d_sz, HPC * DH], f32, kind="ExternalInput")
    wo = nc.dram_tensor("wo", [HPC * DH, d_sz], f32, kind="ExternalInput")
    cos = nc.dram_tensor("cos", [DH, t_sz], bf16, kind="ExternalInput")
    sin = nc.dram_tensor("sin", [DH, t_sz], bf16, kind="ExternalInput")
    msk = nc.dram_tensor("msk", [TK, TK], f32, kind="ExternalInput")
    idn = nc.dram_tensor("idn", [128, 128], f32, kind="ExternalInput")
    one = nc.dram_tensor("one", [128, 128], f32, kind="ExternalInput")
    y = nc.dram_tensor("y", [b_sz, t_sz, d_sz], bf16, kind="ExternalOutput")

    xT_r = xT.ap().rearrange("b (dc p) t -> b p dc t", p=128)
    wq_r = wq.ap().rearrange("(dc p) n -> p dc n", p=128)
    wk_r = wk.ap().rearrange("(dc p) n -> p dc n", p=128)
    wv_r = wv.ap().rearrange("(dc p) n -> p dc n", p=128)
    wo_r = wo.ap().rearrange("(h p) n -> p h n", p=128)
    y_r = y.ap()

    with tile.TileContext(nc) as tc:
        with (
            tc.tile_pool(name="consts", bufs=1) as consts,
            tc.tile_pool(name="wpool", bufs=1) as wpool,
            tc.tile_pool(name="qkv", bufs=1) as qkv,
            tc.tile_pool(name="xpool", bufs=3) as xpool,
            tc.tile_pool(name="rope", bufs=2) as rope,
            tc.tile_pool(name="pex", bufs=4) as pexp,
            tc.tile_pool(name="gpq", bufs=2) as gpq,
            tc.tile_pool(name="nrm", bufs=2) as nrmp,
            tc.tile_pool(name="otn", bufs=6) as otnp,
            tc.tile_pool(name="ysb", bufs=6) as ysbp,
            tc.tile_pool(name="psS", bufs=2, space="PSUM") as psS,
            tc.tile_pool(name="psO", bufs=2, space="PSUM") as psO,
            tc.tile_pool(name="psR", bufs=1, space="PSUM") as psR,
            tc.tile_pool(name="psY", bufs=2, space="PSUM") as psY,
        ):
            cos_sb = consts.tile([DH, t_sz], bf16)
            sin_sb = consts.tile([DH, t_sz], bf16)
            msk_sb = consts.tile([TK, TK], f32r)
            idn_sb = consts.tile([128, 128], f32r)
            # single [128,128] ones tile: column 0 is the sum-matmul
            # stationary; row 32*h is head h's reciprocal-broadcast
            # stationary (at base partition 32*h so the K=1 broadcast
            # matmuls of the two heads land in different PE row groups).
            onesq_sb = consts.tile([128, 128], f32r)

            wq_sb = wpool.tile([128, DC, HPC * DH], f32r)
            wk_sb = wpool.tile([128, DC, HPC * DH], f32r)
            wv_sb = wpool.tile([128, DC, HPC * DH], f32r)
            wo_sb = wpool.tile([128, HPC, d_sz], f32r)

            def load_x_half(xh, b, half, tsl):
                for dc in range(DCH):
                    nc.sync.dma_start(
                        xh[:, dc, :],
                        xT_r[b, :, half * DCH + dc, tsl].bitcast(f32r))

            # first-needed data first: the first x half-tile and q weight
            # chunks feed the very first matmuls, so their DMAs go at the
            # head of every queue; wk/wv/cos/sin follow in consumption
            # order.
            xt_first = [xpool.tile([128, DCH, TT], f32r, tag="xt",
                                   name="xt_first") for _ in range(2)]
            for half in range(2):
                for dc in range(DCH):
                    nc.sync.dma_start(
                        xt_first[half][:, dc, :],
                        xT_r[0, :, half * DCH + dc, 0:TT].bitcast(f32r))
                    nc.sync.dma_start(
                        wq_sb[:, half * DCH + dc, :],
                        wq_r[:, half * DCH + dc, :].bitcast(f32r))
            nc.sync.dma_start(cos_sb[:, 0:TT], cos.ap()[:, 0:TT])
            nc.sync.dma_start(sin_sb[:, 0:TT], sin.ap()[:, 0:TT])
            for dc in range(DC):
                nc.sync.dma_start(wk_sb[:, dc, :],
                                  wk_r[:, dc, :].bitcast(f32r))
            for dc in range(DC):
                nc.sync.dma_start(wv_sb[:, dc, :],
                                  wv_r[:, dc, :].bitcast(f32r))
            # batch 0 is DMA-bound: prefetch the lo half of its second x
            # tile right after the weights (only one ring slot is free -
            # prefetching the hi half would head-of-line block the consts
            # behind it in its DMA queue)
            xt_b0t1 = xpool.tile([128, DCH, TT], f32r, tag="xt",
                                 name="xt_b0t1")
            load_x_half(xt_b0t1, 0, 0, slice(TT, 2 * TT))

            def load_consts():
                # emitted after the first x tile's DMAs: nothing here is
                # needed before attention of the first tile
                for i in range(1, t_sz // TT):
                    sl = slice(i * TT, (i + 1) * TT)
                    nc.sync.dma_start(cos_sb[:, sl], cos.ap()[:, sl])
                    nc.sync.dma_start(sin_sb[:, sl], sin.ap()[:, sl])
                nc.sync.dma_start(msk_sb[:], msk.ap().bitcast(f32r))
                nc.sync.dma_start(idn_sb[:], idn.ap().bitcast(f32r))
                nc.sync.dma_start(onesq_sb[:], one.ap().bitcast(f32r))

            def load_wo():
                # deferred past all of batch 0's x tiles so the 8.4MB of wo
                # doesn't sit ahead of them in the DMA queue FIFOs; first
                # needed by the first out-projection, ~25us into phase B
                for hh in range(HPC):
                    for nch in range(d_sz // 512):
                        nsl = slice(nch * 512, (nch + 1) * 512)
                        nc.sync.dma_start(wo_sb[:, hh, nsl],
                                          wo_r[:, hh, nsl].bitcast(f32r))

            pending = []
            norm_pending = []
            otn_tiles = {}
            xt_prefetch = {(0, 1, 0): xt_b0t1}

            for b in range(b_sz):
                # ------- merged phases: proj tile tt, then attention for
                # q-group qi=tt (needs only qT/kT/vv from tiles <= tt).
                # Attention work overlaps the x/weight DMA of later tiles,
                # which hides batch 0's DMA-bound start and removes the
                # phase/batch boundary stalls.
                kT = [qkv.tile([DH, t_sz], f32r, tag=f"kT{h}", name=f"kT{h}")
                      for h in range(HPC)]
                vv = qkv.tile([128, NKT, HPC * DH], f32r, tag="vv",
                              name="vv")

                for tt in range(t_sz // TT):
                    tsl = slice(tt * TT, (tt + 1) * TT)
                    if b == 0 and tt == 0:
                        xt = xt_first
                        load_consts()
                    else:
                        xt = []
                        for half in range(2):
                            if (b, tt, half) in xt_prefetch:
                                xt.append(xt_prefetch.pop((b, tt, half)))
                            else:
                                xh = xpool.tile([128, DCH, TT], f32r,
                                                tag="xt", name="xt")
                                load_x_half(xh, b, half, tsl)
                                xt.append(xh)

                    # q is only read by this tile's own q-group (qi == tt),
                    # so it lives in a small per-tile ring instead of a
                    # full [DH, T] buffer (frees SBUF for the x prefetch)
                    qTt = [qkv.tile([DH, TT], f32r, tag=f"qT{h}", bufs=2,
                                    name=f"qT{h}") for h in range(HPC)]
                    for h in range(HPC):
                        hs = slice(h * DH, (h + 1) * DH)
                        for dst, w_sb in ((qTt[h][:, :], wq_sb),
                                          (kT[h][:, tsl], wk_sb)):
                            pp = psS.tile([128, TT], f32, tag="st", name="pp")
                            for dc in range(DC):
                                nc.tensor.matmul(
                                    pp[:],
                                    w_sb[:, dc, hs],
                                    xt[dc // DCH][:, dc % DCH, :],
                                    start=(dc == 0), stop=(dc == DC - 1),
                                )
                            # RoPE: dst = ppc*cos + swap(ppc)*sin_signed.
                            # The pp PSUM bank is freed by a fast ACT copy
                            # (if the DVE reads pp directly, the st-ring
                            # couples the PE to DVE backlog). The
                            # rotate-half partition swap runs on the DMA
                            # engine (the DVE cannot pair SBUF operands at
                            # different start partitions).
                            ppc = rope.tile([128, TT], bf16, tag="ppc",
                                            name="ppc")
                            nc.scalar.copy(ppc[:], pp[:])
                            psw = rope.tile([128, TT], bf16, tag="psw",
                                            name="psw")
                            nc.sync.dma_start(psw[0:64, :], ppc[64:128, :])
                            nc.sync.dma_start(psw[64:128, :], ppc[0:64, :])
                            sh = rope.tile([128, TT], bf16, tag="sh",
                                           bufs=2, name="sh")
                            nc.vector.tensor_mul(sh[:], psw[:],
                                                 sin_sb[:, tsl])
                            nc.vector.tensor_mul(dst, ppc[:],
                                                 cos_sb[:, tsl])
                            nc.vector.tensor_add(dst, dst, sh[:])

                    for ts2 in range(TT // TK):
                        vp = psS.tile([TK, HPC * DH], f32, tag="st",
                                      name="vp")
                        for dc in range(DC):
                            nc.tensor.matmul(
                                vp[:],
                                xt[dc // DCH][:, dc % DCH,
                                              ts2 * TK:(ts2 + 1) * TK],
                                wv_sb[:, dc, :],
                                start=(dc == 0), stop=(dc == DC - 1),
                            )
                        kv_i = tt * (TT // TK) + ts2
                        nc.scalar.copy(vv[:, kv_i, :], vp[:])

                    # prefetch one x half-tile ahead (exactly one ring
                    # slot is free), so its DMA sits ahead of the y-output
                    # traffic of the attention below
                    nt_b, nt_tt = (b, tt + 1) if tt + 1 < t_sz // TT \
                        else (b + 1, 0)
                    if nt_b < b_sz and (nt_b, nt_tt) != (0, 1):
                        xh = xpool.tile([128, DCH, TT], f32r, tag="xt",
                                        name="xt")
                        load_x_half(xh, nt_b, 0,
                                    slice(nt_tt * TT, (nt_tt + 1) * TT))
                        xt_prefetch[(nt_b, nt_tt, 0)] = xh

                    # ---------------- attention + out proj for qi=tt ----
                    # both heads interleave within the q-group: every
                    # cross-engine chain (exp -> PV) gets two kv-tiles of
                    # PE work as cover. The causal mask is an identity-
                    # matmul accumulation of an additive -1e30 master
                    # slice into the score PSUM (PE-side: no cross-engine
                    # latency on the exp input).
                    qi = tt
                    nkv = KPG * (qi + 1)
                    outp = [psO.tile([DH, TQ], f32, tag="outT",
                                     name=f"outp{h}") for h in range(HPC)]
                    sump = [psR.tile([1, TQ], f32, tag=f"sums{h}",
                                     name=f"sump{h}") for h in range(HPC)]
                    # off-diagonal exp tiles are pre-summed in quads of 4
                    # on the otherwise-idle GpSimd engine; only one ones-
                    # matmul per quad reaches the PE (flushed at group
                    # end, where the 4 trailing diagonal tiles give the
                    # GpSimd add chain cover). Each group's off-diagonal
                    # count 4*qi is an exact multiple of 4.
                    stash = [None] * HPC
                    qac = [None] * HPC
                    quad_flush = []
                    prev = None
                    for ki in range(nkv):
                        # column-restrict diagonal tiles: kv tile at
                        # diagonal offset dg only attends queries >= dg*TK
                        # within the group; everything below is fully
                        # masked, so score/exp/PV/sum skip those columns
                        # and the mask shrinks to the TK x TK triangle.
                        dg = ki - KPG * qi
                        w0 = 0 if dg < 0 else dg * TK
                        pex = []
                        for h in range(HPC):
                            stp = psS.tile([TK, TQ], f32, tag="st",
                                           name="stp")
                            nc.tensor.matmul(
                                stp[:, w0:],
                                kT[h][:, ki * TK:(ki + 1) * TK],
                                qTt[h][:, w0:],
                                start=True, stop=(dg < 0),
                            )
                            if dg >= 0:
                                nc.tensor.matmul(
                                    stp[:, w0:w0 + TK], idn_sb[:],
                                    msk_sb[:],
                                    start=False, stop=True,
                                )
                            px = pexp.tile([TK, TQ], f32r, tag="pex",
                                           name="pex")
                            nc.scalar.activation(px[:, w0:], stp[:, w0:],
                                                 EXP)
                            pex.append(px)
                            # off-diagonal quad pre-sum on GpSimd (runs in
                            # parallel with the PE; emitted here so the
                            # pex ring slot's reader exists before the
                            # slot is recycled two iterations later)
                            if dg < 0:
                                if ki % 4 == 0:
                                    stash[h] = px
                                elif ki % 4 == 1:
                                    qac[h] = gpq.tile([128, TQ], f32r,
                                                      tag=f"qac{h}",
                                                      bufs=3,
                                                      name=f"qac{h}")
                                    nc.gpsimd.tensor_add(
                                        qac[h][:], stash[h][:], px[:])
                                else:
                                    nc.gpsimd.tensor_add(
                                        qac[h][:], qac[h][:], px[:])
                                    if ki % 4 == 3:
                                        quad_flush.append((h, qac[h]))
                        # PV+sum of the PREVIOUS kv tile go to the PE now,
                        # so the exps hide under the next scores
                        if prev is not None:
                            pki, pw0, ppex = prev
                            pdg = pki - KPG * qi
                            for h in range(HPC):
                                nc.tensor.matmul(
                                    outp[h][:, pw0:],
                                    vv[:, pki, h * DH:(h + 1) * DH],
                                    ppex[h][:, pw0:], start=(pki == 0),
                                    stop=False,
                                )
                            if pdg >= 0:
                                for h in range(HPC):
                                    nc.tensor.matmul(
                                        sump[h][:, pw0:], onesq_sb[:, 0:1],
                                        ppex[h][:, pw0:],
                                        start=(pdg == 0), stop=False,
                                    )
                        prev = (ki, w0, pex)
                        # norm chains pop at ki 2..3 (not 0..1): their
                        # [1,TQ] LN/EXP table ops then never delay the new
                        # group's first exps, which gate the score-psum
                        # ring reuse at the boundary
                        if 2 <= ki < 2 + HPC and norm_pending:
                            norm_pending.pop(0)()
                        if len(pending) > 4:
                            pending.pop(0)()
                    pki, pw0, ppex = prev
                    for h in range(HPC):
                        nc.tensor.matmul(
                            outp[h][:, pw0:],
                            vv[:, pki, h * DH:(h + 1) * DH],
                            ppex[h][:, pw0:], start=(pki == 0), stop=True,
                        )
                    for h in range(HPC):
                        nc.tensor.matmul(
                            sump[h][:, pw0:], onesq_sb[:, 0:1],
                            ppex[h][:, pw0:],
                            start=(pki - KPG * qi == 0), stop=(qi == 0),
                        )
                    for i, (h, qa) in enumerate(quad_flush):
                        nc.tensor.matmul(
                            sump[h][:], onesq_sb[:, 0:1], qa[:],
                            start=False,
                            stop=(i >= len(quad_flush) - HPC),
                        )

                    # copy the unnormalized attention output to SBUF right
                    # away: this frees the psO bank after one ACT op, so
                    # the next group's first PV matmul never waits for the
                    # (deferred, ~3.5us) normalization chain. The chain
                    # then scales otn in place.
                    # (on DVE, not ACT: an ACT copy here sits in the
                    # in-order ACT queue between this group's exps and the
                    # next group's, delaying the exp that frees the score
                    # psum ring)
                    otn_now = []
                    for h in range(HPC):
                        otn = otnp.tile([DH, TQ], f32r, tag="otn",
                                        name="otn")
                        nc.vector.tensor_copy(otn[:], outp[h][:])
                        otn_now.append(otn)

                    # normalization chains, deferred one kv-tile into the
                    # NEXT q-group so their ACT ops (LN, exp(-x)) never
                    # sit between consecutive exps at a group boundary.
                    # rcp = exp(-ln(den)) on ACT (same table set as
                    # Exp/Copy: no ACT_TABLE_LOAD); a ones-matmul
                    # broadcasts it across partitions into a psY-ring
                    # bank.
                    for h in range(HPC):
                        def norm_group(h=h, qi=qi, b=b, otn=otn_now[h],
                                       sump=sump[h]):
                            lnd = nrmp.tile([1, TQ], f32, tag="lnd",
                                            bufs=1, name="lnd")
                            nc.scalar.activation(lnd[:], sump[:], LN)
                            rcp1 = nrmp.tile([1, TQ], f32r, tag="rcp1",
                                             bufs=1, name="rcp1")
                            nc.scalar.activation(rcp1[:], lnd[:], EXP,
                                                 scale=-1.0)
                            rbc = psY.tile([DH, TQ], f32, tag="y",
                                           name="rbc")
                            nc.tensor.matmul(rbc[:],
                                             onesq_sb[0:1, :],
                                             rcp1[:], start=True,
                                             stop=True)
                            # rbs cast on ACT, not DVE: it frees the psY
                            # bank the next out-proj chunk needs, and the
                            # DVE FIFO (otn copies + ysb casts) starts it
                            # ~2us late, stalling the PE
                            rbs = nrmp.tile([DH, TQ], bf16, tag="rbs",
                                            name="rbs")
                            nc.scalar.copy(rbs[:], rbc[:])
                            nc.vector.tensor_mul(otn[:], otn[:], rbs[:])
                            otn_tiles[(h, qi)] = otn

                        norm_pending.append(norm_group)

                    def out_chunk(qi=qi, b=b):
                        def chunk(tc2, qi=qi, b=b):
                            tq0 = qi * TQ + tc2 * TK
                            for nch in range(d_sz // 512):
                                yp = psY.tile([TK, 512], f32,
                                              tag="y", name="yp")
                                for hh in range(HPC):
                                    nc.tensor.matmul(
                                        yp[:],
                                        otn_tiles[(hh, qi)][
                                            :, tc2 * TK:(tc2 + 1) * TK],
                                        wo_sb[:, hh,
                                              nch * 512:(nch + 1) * 512],
                                        start=(hh == 0),
                                        stop=(hh == HPC - 1),
                                    )
                                ysb = ysbp.tile([TK, 512], bf16,
                                                tag="ysb", name="ysb")
                                nc.vector.tensor_copy(ysb[:], yp[:])
                                nc.sync.dma_start(
                                    y_r[b, tq0:tq0 + TK,
                                        nch * 512:(nch + 1) * 512],
                                    ysb[:])
                        return [lambda tc2=tc2: chunk(tc2)
                                for tc2 in range(TQ // TK)]

                    pending.extend(out_chunk())

                    if b == 0 and tt == 1:
                        load_wo()

            for fn in norm_pending:
                fn()
            for fn in pending:
                fn()
    if legalize:
        _legalize_waits(nc, mybir)
    return nc


_NC_CACHE = {}
LAST_RESULT = None


def _get_nc(b_sz, t_sz, d_sz):
    key = (b_sz, t_sz, d_sz)
    if key not in _NC_CACHE:
        _NC_CACHE[key] = _build_nc(b_sz, t_sz, d_sz)
    return _NC_CACHE[key]


def kernel(x, w_q, w_k, w_v, w_o):
    import ml_dtypes
    from concourse.bass_utils import run_bass_kernel_spmd

    BF = ml_dtypes.bfloat16
    b_sz, t_sz, d_sz = x.shape
    scale = np.float32(1.0 / np.sqrt(DH))

    xT = np.ascontiguousarray(np.asarray(x, np.float32).transpose(0, 2, 1))
    w_q = np.asarray(w_q, np.float32)
    w_k = np.asarray(w_k, np.float32)
    w_v = np.asarray(w_v, np.float32)
    w_o = np.asarray(w_o, np.float32)
    cosT, sinT = _rope_tables(t_sz, DH, THETA)
    mask = _causal_mask_add(TK)

    in_maps = []
    for c in range(NCORES):
        cs = slice(c * HPC * DH, (c + 1) * HPC * DH)
        in_maps.append({
            "xT": xT,
            "wq": np.ascontiguousarray(w_q[:, cs] * scale),
            "wk": np.ascontiguousarray(w_k[:, cs]),
            "wv": np.ascontiguousarray(w_v[:, cs]),
            "wo": np.ascontiguousarray(w_o[cs, :]),
            "cos": cosT.astype(BF),
            "sin": sinT.astype(BF),
            "msk": mask,
            "idn": np.eye(128, dtype=np.float32),
            "one": np.ones((128, 128), np.float32),
        })

    nc = _get_nc(b_sz, t_sz, d_sz)
    res = run_bass_kernel_spmd(nc, in_maps, core_ids=list(range(NCORES)))
    global LAST_RESULT
    LAST_RESULT = res

    out = np.asarray(res.results[0]["y"]).astype(np.float32)
    for c in range(1, NCORES):
        out += np.asarray(res.results[c]["y"]).astype(np.float32)
    return out



# revision 60
# speedup vs baseline: 1.0142x; 1.0142x over previous
"""Causal self-attention with RoPE on 8 Trainium2 NeuronCores.

Sharding: Megatron-style head parallelism. 16 heads / 8 cores = 2 heads per
core. Each core computes q/k/v projections for its 2 heads (column-parallel),
full causal attention for those heads, and a partial output projection
(row-parallel slice of w_o). The host sums the 8 partial outputs.

On-chip layout: everything transposed. Host passes xT = x^T per batch
[B, D, T]; projections produce qT/kT [dh, t] directly (lhsT = weight slice,
rhs = xT chunk) and v [t, dh] (lhsT = xT chunk, rhs = w_v slice). Scores are
computed transposed, ST[kv, q] = matmul(lhsT=kT_chunk, rhs=qT_group), which
makes P^T directly usable as the moving operand of the PV matmul - no
on-chip transposes anywhere. The causal mask is accumulated onto the score
PSUM by an identity-matmul against an additive -1e30 mask slice (PE-side:
no cross-engine latency on the exp input).

All matmul operands are float32r (full PE rate at moving-dim >= 256, and -
unlike bf16 - the fp32r matmul self-loads its stationary, overlapping the
weight load with the stream; bf16 operands emit separate LDWEIGHTS
instructions that serialize ~100ns+ per dependent matmul, measured; the
walrus verifier also forbids mixing f32r with bf16 matmul operands).

Schedule (the changes that took the 952us baseline to ~800us):
- Merged phases: attention for q-group qi=tt runs right after projection
  tile tt (it only needs qT/kT/vv from tiles <= tt), so attention hides
  batch 0's DMA-bound start and the phase/batch boundary stalls.
- Column-restricted diagonal tiles: the kv tile at diagonal offset dg
  only attends queries >= dg*TK within its group, so score/exp/PV/sum
  all skip the fully-masked columns, and the additive causal mask
  shrinks to one TK x TK triangle applied by a 128-col identity matmul
  (measured ~-37us of PE busy vs full-width diagonals).
- Both heads interleave within each q-group, and PV + denominator-sum
  matmuls are emitted one kv-tile BEHIND the scores: the ACT exp latency
  (~650ns) hides under two kv-tiles (~1.4us) of PE work.
- Off-diagonal exp tiles are pre-summed in quads of 4 on the otherwise
  idle GpSimd engine; one ones-matmul per quad (flushed at group end,
  covered by the diagonal span) replaces four, quartering the PE cost
  of the softmax denominators for the off-diagonal bulk.
- The unnormalized attention output is copied PSUM->SBUF by one ACT op
  at group end, freeing the psO bank immediately (the 2-deep psO ring
  otherwise makes the next group's first PV wait out the ~3.5us
  normalization chain); the chain then scales that copy in place.
- The softmax reciprocal is exp(-ln(den)) on ACT [1,TQ] (Ln/Exp/Copy share
  one activation table set, so no ACT_TABLE_LOAD; the baseline's DVE
  reciprocal on [128,TQ] cost 3.3us x32 = 107us of DVE). It is deferred
  one kv-tile into the next group so the LN/exp never sit between
  consecutive exps in the in-order ACT queue; a ones-matmul broadcasts
  the reciprocal across partitions into a psY-ring bank.
- RoPE reads a fast ACT copy of the projection PSUM (if the DVE reads the
  PSUM directly, the 2-bank st-ring couples the PE to DVE backlog:
  measured ~3.9us stalls plus p-state resets); the rotate-half partition
  swap runs on the DMA engine (the DVE cannot pair SBUF operands at
  different start partitions); the multiply/add run in bf16.
- The out-projection is emitted in 4 q-chunks popped between kv tiles of
  LATER groups (a 16-copy ysb burst ahead of the next exps otherwise
  stalls the PE via the in-order DVE/ACT queues); output y is written in
  bf16 (halves output DMA); the host upcasts and sums the 8 partials.
- TT=512 projections (half the matmul instruction count of TT=256); x
  tiles are split into two half-depth chunks to fit SBUF, with a one-slot
  DMA prefetch ahead of the attention's y-output traffic; wo loads are
  deferred behind batch 0's x tiles; q lives in a small per-tile ring
  (only its own q-group ever reads it).

Measured dead ends (don't revisit): walrus rejects matmul PSUM dst
partition offsets (s3d3_mm_valid_dst_partition), so col-tiled
concurrent M=1 sum matmuls at partitions 32/64/96 won't compile;
x-in-bf16 with on-chip upcast loses ~130us (GpSimd/ACT copies stall
the projections; GpSimd tensor_copy of a [128,4,512] block measures
~7us); splitting startup DMA chunks in half doubles descriptor count
and makes the DMA-queue-limited startup WORSE.

The attention scale 1/sqrt(dh) is folded into w_q on the host. No
max-subtraction: logits are q.k/sqrt(dh) with unit-ish variance,
|logit| << 88, identical math to the reference.
"""

import numpy as np

B, T, D = 4, 2048, 2048
H, DH = 16, 128
NCORES = 8
HPC = H // NCORES  # heads per core
THETA = 10000.0

TT = 512  # projection t-tile (moving dim of q/k projection matmuls)
TQ = 512  # attention q-group width
TK = 128  # kv tile (contraction chunk of PV / partition dim of ST)


def _rope_tables(seq_len, d_head, theta):
    # Matches reference.rope_cos_sin numerics, then transposes to [dh, t]
    # and folds the rotate-half sign into sin.
    inv_freq = 1.0 / (theta ** (np.arange(0, d_head, 2, dtype=np.float32) / d_head))
    t = np.arange(seq_len, dtype=np.float32)
    freqs = np.einsum("i,j->ij", t, inv_freq)
    emb = np.concatenate([freqs, freqs], axis=-1)  # [T, dh]
    cosT = np.ascontiguousarray(np.cos(emb).astype(np.float32).T)  # [dh, T]
    sinT = np.ascontiguousarray(np.sin(emb).astype(np.float32).T)
    sgn = np.ones((d_head, 1), np.float32)
    sgn[: d_head // 2] = -1.0
    return cosT, sinT * sgn


def _causal_mask_add(tk):
    # Additive causal triangle [tk, tk]: with column-restricted diagonal
    # tiles the only region that ever needs masking is the tk x tk block
    # on the diagonal itself: entry is -1e30 (masked) iff c < r.
    m = np.zeros((tk, tk), np.float32)
    for r in range(tk):
        m[r, :r] = -1e30
    return m


def _legalize_waits(nc, mybir):
    """Walrus on this toolchain refuses more than one embedded sync wait
    per engine instruction. Hoist extra waits into standalone
    EventSemaphore instructions on the same engine queue (the sequencer
    executes them in-stream before the instruction, same gating)."""
    n = 0
    for f in nc.m.functions:
        for bb in f.blocks:
            out = []
            for inst in bb.instructions:
                si = inst.sync_info
                if (si and si.on_wait and len(si.on_wait) > 1
                        and not isinstance(inst, mybir.InstEventSemaphore)):
                    for w in si.on_wait[:-1]:
                        out.append(mybir.InstEventSemaphore(
                            name=f"WH-{n}", engine=inst.engine,
                            sync_info=mybir.SyncInfo(
                                on_wait=[w], on_update=[])))
                        n += 1
                    inst.sync_info = mybir.SyncInfo(
                        on_wait=[si.on_wait[-1]],
                        on_update=list(si.on_update))
                out.append(inst)
            bb.instructions = out
    return n


def _build_nc(b_sz, t_sz, d_sz, legalize=True):
    import concourse.bass as bass
    import concourse.tile as tile
    from concourse import mybir

    f32 = mybir.dt.float32
    f32r = mybir.dt.float32r
    bf16 = mybir.dt.bfloat16
    EXP = mybir.ActivationFunctionType.Exp
    LN = mybir.ActivationFunctionType.Ln

    DC = d_sz // 128         # contraction chunks
    DCH = DC // 2            # chunks per x half-tile
    NQG = t_sz // TQ         # q groups per (batch, head)
    NKT = t_sz // TK         # kv tiles
    KPG = TQ // TK           # kv tiles per q group (diagonal span)

    nc = bass.Bass("TRN2", target_bir_lowering=False, debug=False,
                   enable_asserts=False, dynamic_dma_scratch_size=2048)

    xT = nc.dram_tensor("xT", [b_sz, d_sz, t_sz], f32, kind="ExternalInput")
    wq = nc.dram_tensor("wq", [d_sz, HPC * DH], f32, kind="ExternalInput")
    wk = nc.dram_tensor("wk", [d_sz, HPC * DH], f32, kind="ExternalInput")
    wv = nc.dram_tensor("wv", [d_sz, HPC * DH], f32, kind="ExternalInput")
    wo = nc.dram_tensor("wo", [HPC * DH, d_sz], f32, kind="ExternalInput")
    cos = nc.dram_tensor("cos", [DH, t_sz], bf16, kind="ExternalInput")
    sin = nc.dram_tensor("sin", [DH, t_sz], bf16, kind="ExternalInput")
    msk = nc.dram_tensor("msk", [TK, TK], f32, kind="ExternalInput")
    idn = nc.dram_tensor("idn", [128, 128], f32, kind="ExternalInput")
    one = nc.dram_tensor("one", [128, 128], f32, kind="ExternalInput")
    y = nc.dram_tensor("y", [b_sz, t_sz, d_sz], bf16, kind="ExternalOutput")

    xT_r = xT.ap().rearrange("b (dc p) t -> b p dc t", p=128)
    wq_r = wq.ap().rearrange("(dc p) n -> p dc n", p=128)
    wk_r = wk.ap().rearrange("(dc p) n -> p dc n", p=128)
    wv_r = wv.ap().rearrange("(dc p) n -> p dc n", p=128)
    wo_r = wo.ap().rearrange("(h p) n -> p h n", p=128)
    y_r = y.ap()

    with tile.TileContext(nc) as tc:
        with (
            tc.tile_pool(name="consts", bufs=1) as consts,
            tc.tile_pool(name="wpool", bufs=1) as wpool,
            tc.tile_pool(name="qkv", bufs=1) as qkv,
            tc.tile_pool(name="xpool", bufs=3) as xpool,
            tc.tile_pool(name="rope", bufs=2) as rope,
            tc.tile_pool(name="pex", bufs=4) as pexp,
            tc.tile_pool(name="gpq", bufs=2) as gpq,
            tc.tile_pool(name="nrm", bufs=2) as nrmp,
            tc.tile_pool(name="otn", bufs=6) as otnp,
            tc.tile_pool(name="ysb", bufs=8) as ysbp,
            tc.tile_pool(name="psS", bufs=2, space="PSUM") as psS,
            tc.tile_pool(name="psO", bufs=2, space="PSUM") as psO,
            tc.tile_pool(name="psR", bufs=1, space="PSUM") as psR,
            tc.tile_pool(name="psY", bufs=2, space="PSUM") as psY,
        ):
            cos_sb = consts.tile([DH, t_sz], bf16)
            sin_sb = consts.tile([DH, t_sz], bf16)
            msk_sb = consts.tile([TK, TK], f32r)
            idn_sb = consts.tile([128, 128], f32r)
            # single [128,128] ones tile: column 0 is the sum-matmul
            # stationary; row 32*h is head h's reciprocal-broadcast
            # stationary (at base partition 32*h so the K=1 broadcast
            # matmuls of the two heads land in different PE row groups).
            onesq_sb = consts.tile([128, 128], f32r)

            wq_sb = wpool.tile([128, DC, HPC * DH], f32r)
            wk_sb = wpool.tile([128, DC, HPC * DH], f32r)
            wv_sb = wpool.tile([128, DC, HPC * DH], f32r)
            wo_sb = wpool.tile([128, HPC, d_sz], f32r)

            def load_x_half(xh, b, half, tsl):
                for dc in range(DCH):
                    nc.sync.dma_start(
                        xh[:, dc, :],
                        xT_r[b, :, half * DCH + dc, tsl].bitcast(f32r))

            # first-needed data first: the first x half-tile and q weight
            # chunks feed the very first matmuls, so their DMAs go at the
            # head of every queue; wk/wv/cos/sin follow in consumption
            # order.
            xt_first = [xpool.tile([128, DCH, TT], f32r, tag="xt",
                                   name="xt_first") for _ in range(2)]
            for half in range(2):
                for dc in range(DCH):
                    nc.sync.dma_start(
                        xt_first[half][:, dc, :],
                        xT_r[0, :, half * DCH + dc, 0:TT].bitcast(f32r))
                    nc.sync.dma_start(
                        wq_sb[:, half * DCH + dc, :],
                        wq_r[:, half * DCH + dc, :].bitcast(f32r))
            nc.sync.dma_start(cos_sb[:, 0:TT], cos.ap()[:, 0:TT])
            nc.sync.dma_start(sin_sb[:, 0:TT], sin.ap()[:, 0:TT])
            for dc in range(DC):
                nc.sync.dma_start(wk_sb[:, dc, :],
                                  wk_r[:, dc, :].bitcast(f32r))
            for dc in range(DC):
                nc.sync.dma_start(wv_sb[:, dc, :],
                                  wv_r[:, dc, :].bitcast(f32r))
            # batch 0 is DMA-bound: prefetch the lo half of its second x
            # tile right after the weights (only one ring slot is free -
            # prefetching the hi half would head-of-line block the consts
            # behind it in its DMA queue)
            xt_b0t1 = xpool.tile([128, DCH, TT], f32r, tag="xt",
                                 name="xt_b0t1")
            load_x_half(xt_b0t1, 0, 0, slice(TT, 2 * TT))

            def load_consts():
                # emitted after the first x tile's DMAs: nothing here is
                # needed before attention of the first tile
                for i in range(1, t_sz // TT):
                    sl = slice(i * TT, (i + 1) * TT)
                    nc.sync.dma_start(cos_sb[:, sl], cos.ap()[:, sl])
                    nc.sync.dma_start(sin_sb[:, sl], sin.ap()[:, sl])
                nc.sync.dma_start(msk_sb[:], msk.ap().bitcast(f32r))
                nc.sync.dma_start(idn_sb[:], idn.ap().bitcast(f32r))
                nc.sync.dma_start(onesq_sb[:], one.ap().bitcast(f32r))

            def load_wo():
                # deferred past all of batch 0's x tiles so the 8.4MB of wo
                # doesn't sit ahead of them in the DMA queue FIFOs; first
                # needed by the first out-projection, ~25us into phase B
                for hh in range(HPC):
                    for nch in range(d_sz // 512):
                        nsl = slice(nch * 512, (nch + 1) * 512)
                        nc.sync.dma_start(wo_sb[:, hh, nsl],
                                          wo_r[:, hh, nsl].bitcast(f32r))

            pending = []
            norm_pending = []
            otn_tiles = {}
            xt_prefetch = {(0, 1, 0): xt_b0t1}

            for b in range(b_sz):
                # ------- merged phases: proj tile tt, then attention for
                # q-group qi=tt (needs only qT/kT/vv from tiles <= tt).
                # Attention work overlaps the x/weight DMA of later tiles,
                # which hides batch 0's DMA-bound start and removes the
                # phase/batch boundary stalls.
                kT = [qkv.tile([DH, t_sz], f32r, tag=f"kT{h}", name=f"kT{h}")
                      for h in range(HPC)]
                vv = qkv.tile([128, NKT, HPC * DH], f32r, tag="vv",
                              name="vv")

                for tt in range(t_sz // TT):
                    tsl = slice(tt * TT, (tt + 1) * TT)
                    if b == 0 and tt == 0:
                        xt = xt_first
                        load_consts()
                    else:
                        xt = []
                        for half in range(2):
                            if (b, tt, half) in xt_prefetch:
                                xt.append(xt_prefetch.pop((b, tt, half)))
                            else:
                                xh = xpool.tile([128, DCH, TT], f32r,
                                                tag="xt", name="xt")
                                load_x_half(xh, b, half, tsl)
                                xt.append(xh)

                    # q is only read by this tile's own q-group (qi == tt),
                    # so it lives in a small per-tile ring instead of a
                    # full [DH, T] buffer (frees SBUF for the x prefetch)
                    qTt = [qkv.tile([DH, TT], f32r, tag=f"qT{h}", bufs=2,
                                    name=f"qT{h}") for h in range(HPC)]
                    for h in range(HPC):
                        hs = slice(h * DH, (h + 1) * DH)
                        for dst, w_sb in ((qTt[h][:, :], wq_sb),
                                          (kT[h][:, tsl], wk_sb)):
                            pp = psS.tile([128, TT], f32, tag="st", name="pp")
                            for dc in range(DC):
                                nc.tensor.matmul(
                                    pp[:],
                                    w_sb[:, dc, hs],
                                    xt[dc // DCH][:, dc % DCH, :],
                                    start=(dc == 0), stop=(dc == DC - 1),
                                )
                            # RoPE: dst = ppc*cos + swap(ppc)*sin_signed.
                            # The pp PSUM bank is freed by a fast ACT copy
                            # (if the DVE reads pp directly, the st-ring
                            # couples the PE to DVE backlog). The
                            # rotate-half partition swap runs on the DMA
                            # engine (the DVE cannot pair SBUF operands at
                            # different start partitions).
                            ppc = rope.tile([128, TT], bf16, tag="ppc",
                                            name="ppc")
                            nc.scalar.copy(ppc[:], pp[:])
                            psw = rope.tile([128, TT], bf16, tag="psw",
                                            name="psw")
                            nc.sync.dma_start(psw[0:64, :], ppc[64:128, :])
                            nc.sync.dma_start(psw[64:128, :], ppc[0:64, :])
                            sh = rope.tile([128, TT], bf16, tag="sh",
                                           bufs=2, name="sh")
                            nc.vector.tensor_mul(sh[:], psw[:],
                                                 sin_sb[:, tsl])
                            nc.vector.tensor_mul(dst, ppc[:],
                                                 cos_sb[:, tsl])
                            nc.vector.tensor_add(dst, dst, sh[:])

                    for ts2 in range(TT // TK):
                        vp = psS.tile([TK, HPC * DH], f32, tag="st",
                                      name="vp")
                        for dc in range(DC):
                            nc.tensor.matmul(
                                vp[:],
                                xt[dc // DCH][:, dc % DCH,
                                              ts2 * TK:(ts2 + 1) * TK],
                                wv_sb[:, dc, :],
                                start=(dc == 0), stop=(dc == DC - 1),
                            )
                        kv_i = tt * (TT // TK) + ts2
                        nc.scalar.copy(vv[:, kv_i, :], vp[:])

                    # prefetch one x half-tile ahead (exactly one ring
                    # slot is free), so its DMA sits ahead of the y-output
                    # traffic of the attention below
                    nt_b, nt_tt = (b, tt + 1) if tt + 1 < t_sz // TT \
                        else (b + 1, 0)
                    if nt_b < b_sz and (nt_b, nt_tt) != (0, 1):
                        xh = xpool.tile([128, DCH, TT], f32r, tag="xt",
                                        name="xt")
                        load_x_half(xh, nt_b, 0,
                                    slice(nt_tt * TT, (nt_tt + 1) * TT))
                        xt_prefetch[(nt_b, nt_tt, 0)] = xh

                    # ---------------- attention + out proj for qi=tt ----
                    # both heads interleave within the q-group: every
                    # cross-engine chain (exp -> PV) gets two kv-tiles of
                    # PE work as cover. The causal mask is an identity-
                    # matmul accumulation of an additive -1e30 master
                    # slice into the score PSUM (PE-side: no cross-engine
                    # latency on the exp input).
                    qi = tt
                    nkv = KPG * (qi + 1)
                    outp = [psO.tile([DH, TQ], f32, tag="outT",
                                     name=f"outp{h}") for h in range(HPC)]
                    sump = [psR.tile([1, TQ], f32, tag=f"sums{h}",
                                     name=f"sump{h}") for h in range(HPC)]
                    # off-diagonal exp tiles are pre-summed in quads of 4
                    # on the otherwise-idle GpSimd engine; only one ones-
                    # matmul per quad reaches the PE (flushed at group
                    # end, where the 4 trailing diagonal tiles give the
                    # GpSimd add chain cover). Each group's off-diagonal
                    # count 4*qi is an exact multiple of 4.
                    stash = [None] * HPC
                    qac = [None] * HPC
                    quad_flush = []
                    prev = None
                    for ki in range(nkv):
                        # column-restrict diagonal tiles: kv tile at
                        # diagonal offset dg only attends queries >= dg*TK
                        # within the group; everything below is fully
                        # masked, so score/exp/PV/sum skip those columns
                        # and the mask shrinks to the TK x TK triangle.
                        dg = ki - KPG * qi
                        w0 = 0 if dg < 0 else dg * TK
                        pex = []
                        for h in range(HPC):
                            stp = psS.tile([TK, TQ], f32, tag="st",
                                           name="stp")
                            nc.tensor.matmul(
                                stp[:, w0:],
                                kT[h][:, ki * TK:(ki + 1) * TK],
                                qTt[h][:, w0:],
                                start=True, stop=(dg < 0),
                            )
                            if dg >= 0:
                                nc.tensor.matmul(
                                    stp[:, w0:w0 + TK], idn_sb[:],
                                    msk_sb[:],
                                    start=False, stop=True,
                                )
                            px = pexp.tile([TK, TQ], f32r, tag="pex",
                                           name="pex")
                            nc.scalar.activation(px[:, w0:], stp[:, w0:],
                                                 EXP)
                            pex.append(px)
                            # off-diagonal quad pre-sum on GpSimd (runs in
                            # parallel with the PE; emitted here so the
                            # pex ring slot's reader exists before the
                            # slot is recycled two iterations later)
                            if dg < 0:
                                if ki % 4 == 0:
                                    stash[h] = px
                                elif ki % 4 == 1:
                                    qac[h] = gpq.tile([128, TQ], f32r,
                                                      tag=f"qac{h}",
                                                      bufs=3,
                                                      name=f"qac{h}")
                                    nc.gpsimd.tensor_add(
                                        qac[h][:], stash[h][:], px[:])
                                else:
                                    nc.gpsimd.tensor_add(
                                        qac[h][:], qac[h][:], px[:])
                                    if ki % 4 == 3:
                                        quad_flush.append((h, qac[h]))
                        # PV+sum of the PREVIOUS kv tile go to the PE now,
                        # so the exps hide under the next scores
                        if prev is not None:
                            pki, pw0, ppex = prev
                            pdg = pki - KPG * qi
                            for h in range(HPC):
                                nc.tensor.matmul(
                                    outp[h][:, pw0:],
                                    vv[:, pki, h * DH:(h + 1) * DH],
                                    ppex[h][:, pw0:], start=(pki == 0),
                                    stop=False,
                                )
                            if pdg >= 0:
                                for h in range(HPC):
                                    nc.tensor.matmul(
                                        sump[h][:, pw0:], onesq_sb[:, 0:1],
                                        ppex[h][:, pw0:],
                                        start=(pdg == 0), stop=False,
                                    )
                        prev = (ki, w0, pex)
                        # out-proj chunk first (independent PE work whose
                        # psY slot is long freed), THEN the norm chain
                        # whose rbc matmul depends on fresh ACT/DVE work.
                        # Norm chains pop at ki 2..3 (not 0..1): their
                        # [1,TQ] LN/EXP table ops then never delay the new
                        # group's first exps, which gate the score-psum
                        # ring reuse at the boundary
                        if len(pending) > 4:
                            pending.pop(0)()
                        if 2 <= ki < 2 + HPC and norm_pending:
                            norm_pending.pop(0)()
                    pki, pw0, ppex = prev
                    for h in range(HPC):
                        nc.tensor.matmul(
                            outp[h][:, pw0:],
                            vv[:, pki, h * DH:(h + 1) * DH],
                            ppex[h][:, pw0:], start=(pki == 0), stop=True,
                        )
                    for h in range(HPC):
                        nc.tensor.matmul(
                            sump[h][:, pw0:], onesq_sb[:, 0:1],
                            ppex[h][:, pw0:],
                            start=(pki - KPG * qi == 0), stop=(qi == 0),
                        )
                    for i, (h, qa) in enumerate(quad_flush):
                        nc.tensor.matmul(
                            sump[h][:], onesq_sb[:, 0:1], qa[:],
                            start=False,
                            stop=(i >= len(quad_flush) - HPC),
                        )

                    # copy the unnormalized attention output to SBUF right
                    # away: this frees the psO bank after one ACT op, so
                    # the next group's first PV matmul never waits for the
                    # (deferred, ~3.5us) normalization chain. The chain
                    # then scales otn in place.
                    # (on DVE, not ACT: an ACT copy here sits in the
                    # in-order ACT queue between this group's exps and the
                    # next group's, delaying the exp that frees the score
                    # psum ring)
                    otn_now = []
                    for h in range(HPC):
                        otn = otnp.tile([DH, TQ], f32r, tag="otn",
                                        name="otn")
                        nc.vector.tensor_copy(otn[:], outp[h][:])
                        otn_now.append(otn)

                    # normalization chains, deferred one kv-tile into the
                    # NEXT q-group so their ACT ops (LN, exp(-x)) never
                    # sit between consecutive exps at a group boundary.
                    # rcp = exp(-ln(den)) on ACT (same table set as
                    # Exp/Copy: no ACT_TABLE_LOAD); a ones-matmul
                    # broadcasts it across partitions into a psY-ring
                    # bank.
                    for h in range(HPC):
                        def norm_group(h=h, qi=qi, b=b, otn=otn_now[h],
                                       sump=sump[h]):
                            lnd = nrmp.tile([1, TQ], f32, tag="lnd",
                                            bufs=1, name="lnd")
                            nc.scalar.activation(lnd[:], sump[:], LN)
                            rcp1 = nrmp.tile([1, TQ], f32r, tag="rcp1",
                                             bufs=1, name="rcp1")
                            nc.scalar.activation(rcp1[:], lnd[:], EXP,
                                                 scale=-1.0)
                            rbc = psY.tile([DH, TQ], f32, tag="y",
                                           name="rbc")
                            nc.tensor.matmul(rbc[:],
                                             onesq_sb[0:1, :],
                                             rcp1[:], start=True,
                                             stop=True)
                            rbs = nrmp.tile([DH, TQ], bf16, tag="rbs",
                                            name="rbs")
                            nc.vector.tensor_copy(rbs[:], rbc[:])
                            nc.vector.tensor_mul(otn[:], otn[:], rbs[:])
                            otn_tiles[(h, qi)] = otn

                        norm_pending.append(norm_group)

                    def out_chunk(qi=qi, b=b):
                        def chunk(tc2, qi=qi, b=b):
                            tq0 = qi * TQ + tc2 * TK
                            for nch in range(d_sz // 512):
                                yp = psY.tile([TK, 512], f32,
                                              tag="y", name="yp")
                                for hh in range(HPC):
                                    nc.tensor.matmul(
                                        yp[:],
                                        otn_tiles[(hh, qi)][
                                            :, tc2 * TK:(tc2 + 1) * TK],
                                        wo_sb[:, hh,
                                              nch * 512:(nch + 1) * 512],
                                        start=(hh == 0),
                                        stop=(hh == HPC - 1),
                                    )
                                ysb = ysbp.tile([TK, 512], bf16,
                                                tag="ysb", name="ysb")
                                nc.vector.tensor_copy(ysb[:], yp[:])
                                nc.sync.dma_start(
                                    y_r[b, tq0:tq0 + TK,
                                        nch * 512:(nch + 1) * 512],
                                    ysb[:])
                        return [lambda tc2=tc2: chunk(tc2)
                                for tc2 in range(TQ // TK)]

                    pending.extend(out_chunk())

                    if b == 0 and tt == 1:
                        load_wo()

            for fn in norm_pending:
                fn()
            for fn in pending:
                fn()
    if legalize:
        _legalize_waits(nc, mybir)
    return nc


_NC_CACHE = {}
LAST_RESULT = None


def _get_nc(b_sz, t_sz, d_sz):
    key = (b_sz, t_sz, d_sz)
    if key not in _NC_CACHE:
        _NC_CACHE[key] = _build_nc(b_sz, t_sz, d_sz)
    return _NC_CACHE[key]


def kernel(x, w_q, w_k, w_v, w_o):
    import ml_dtypes
    from concourse.bass_utils import run_bass_kernel_spmd

    BF = ml_dtypes.bfloat16
    b_sz, t_sz, d_sz = x.shape
    scale = np.float32(1.0 / np.sqrt(DH))

    xT = np.ascontiguousarray(np.asarray(x, np.float32).transpose(0, 2, 1))
    w_q = np.asarray(w_q, np.float32)
    w_k = np.asarray(w_k, np.float32)
    w_v = np.asarray(w_v, np.float32)
    w_o = np.asarray(w_o, np.float32)
    cosT, sinT = _rope_tables(t_sz, DH, THETA)
    mask = _causal_mask_add(TK)

    in_maps = []
    for c in range(NCORES):
        cs = slice(c * HPC * DH, (c + 1) * HPC * DH)
        in_maps.append({
            "xT": xT,
            "wq": np.ascontiguousarray(w_q[:, cs] * scale),
            "wk": np.ascontiguousarray(w_k[:, cs]),
            "wv": np.ascontiguousarray(w_v[:, cs]),
            "wo": np.ascontiguousarray(w_o[cs, :]),
            "cos": cosT.astype(BF),
            "sin": sinT.astype(BF),
            "msk": mask,
            "idn": np.eye(128, dtype=np.float32),
            "one": np.ones((128, 128), np.float32),
        })

    nc = _get_nc(b_sz, t_sz, d_sz)
    res = run_bass_kernel_spmd(nc, in_maps, core_ids=list(range(NCORES)))
    global LAST_RESULT
    LAST_RESULT = res

    out = np.asarray(res.results[0]["y"]).astype(np.float32)
    for c in range(1, NCORES):
        out += np.asarray(res.results[c]["y"]).astype(np.float32)
    return out



# revision 61
# speedup vs baseline: 1.0290x; 1.0145x over previous
"""Causal self-attention with RoPE on 8 Trainium2 NeuronCores.

Sharding: Megatron-style head parallelism. 16 heads / 8 cores = 2 heads per
core. Each core computes q/k/v projections for its 2 heads (column-parallel),
full causal attention for those heads, and a partial output projection
(row-parallel slice of w_o). The host sums the 8 partial outputs.

On-chip layout: everything transposed. Host passes xT = x^T per batch
[B, D, T]; projections produce qT/kT [dh, t] directly (lhsT = weight slice,
rhs = xT chunk) and v [t, dh] (lhsT = xT chunk, rhs = w_v slice). Scores are
computed transposed, ST[kv, q] = matmul(lhsT=kT_chunk, rhs=qT_group), which
makes P^T directly usable as the moving operand of the PV matmul - no
on-chip transposes anywhere. The causal mask is accumulated onto the score
PSUM by an identity-matmul against an additive -1e30 mask slice (PE-side:
no cross-engine latency on the exp input).

All matmul operands are float32r (full PE rate at moving-dim >= 256, and -
unlike bf16 - the fp32r matmul self-loads its stationary, overlapping the
weight load with the stream; bf16 operands emit separate LDWEIGHTS
instructions that serialize ~100ns+ per dependent matmul, measured; the
walrus verifier also forbids mixing f32r with bf16 matmul operands).

Schedule (the changes that took the 952us baseline to ~800us):
- Merged phases: attention for q-group qi=tt runs right after projection
  tile tt (it only needs qT/kT/vv from tiles <= tt), so attention hides
  batch 0's DMA-bound start and the phase/batch boundary stalls.
- Column-restricted diagonal tiles: the kv tile at diagonal offset dg
  only attends queries >= dg*TK within its group, so score/exp/PV/sum
  all skip the fully-masked columns, and the additive causal mask
  shrinks to one TK x TK triangle applied by a 128-col identity matmul
  (measured ~-37us of PE busy vs full-width diagonals).
- Both heads interleave within each q-group, and PV + denominator-sum
  matmuls are emitted one kv-tile BEHIND the scores: the ACT exp latency
  (~650ns) hides under two kv-tiles (~1.4us) of PE work.
- Off-diagonal exp tiles are pre-summed in quads of 4 on the otherwise
  idle GpSimd engine; one ones-matmul per quad (flushed at group end,
  covered by the diagonal span) replaces four, quartering the PE cost
  of the softmax denominators for the off-diagonal bulk.
- The unnormalized attention output is copied PSUM->SBUF by one ACT op
  at group end, freeing the psO bank immediately (the 2-deep psO ring
  otherwise makes the next group's first PV wait out the ~3.5us
  normalization chain); the chain then scales that copy in place.
- The softmax reciprocal is exp(-ln(den)) on ACT [1,TQ] (Ln/Exp/Copy share
  one activation table set, so no ACT_TABLE_LOAD; the baseline's DVE
  reciprocal on [128,TQ] cost 3.3us x32 = 107us of DVE). It is deferred
  one kv-tile into the next group so the LN/exp never sit between
  consecutive exps in the in-order ACT queue; a ones-matmul broadcasts
  the reciprocal across partitions into a psY-ring bank.
- RoPE reads a fast ACT copy of the projection PSUM (if the DVE reads the
  PSUM directly, the 2-bank st-ring couples the PE to DVE backlog:
  measured ~3.9us stalls plus p-state resets); the rotate-half partition
  swap runs on the DMA engine (the DVE cannot pair SBUF operands at
  different start partitions); the multiply/add run in bf16.
- The out-projection is emitted in 4 q-chunks popped between kv tiles of
  LATER groups (a 16-copy ysb burst ahead of the next exps otherwise
  stalls the PE via the in-order DVE/ACT queues); output y is written in
  bf16 (halves output DMA); the host upcasts and sums the 8 partials.
- TT=512 projections (half the matmul instruction count of TT=256); x
  tiles are split into two half-depth chunks to fit SBUF, with a one-slot
  DMA prefetch ahead of the attention's y-output traffic; wo loads are
  deferred behind batch 0's x tiles; q lives in a small per-tile ring
  (only its own q-group ever reads it).

Measured dead ends (don't revisit): walrus rejects matmul PSUM dst
partition offsets (s3d3_mm_valid_dst_partition), so col-tiled
concurrent M=1 sum matmuls at partitions 32/64/96 won't compile;
x-in-bf16 with on-chip upcast loses ~130us (GpSimd/ACT copies stall
the projections; GpSimd tensor_copy of a [128,4,512] block measures
~7us); splitting startup DMA chunks in half doubles descriptor count
and makes the DMA-queue-limited startup WORSE.

The attention scale 1/sqrt(dh) is folded into w_q on the host. No
max-subtraction: logits are q.k/sqrt(dh) with unit-ish variance,
|logit| << 88, identical math to the reference.
"""

import numpy as np

B, T, D = 4, 2048, 2048
H, DH = 16, 128
NCORES = 8
HPC = H // NCORES  # heads per core
THETA = 10000.0

TT = 512  # projection t-tile (moving dim of q/k projection matmuls)
TQ = 512  # attention q-group width
TK = 128  # kv tile (contraction chunk of PV / partition dim of ST)


def _rope_tables(seq_len, d_head, theta):
    # Matches reference.rope_cos_sin numerics, then transposes to [dh, t]
    # and folds the rotate-half sign into sin.
    inv_freq = 1.0 / (theta ** (np.arange(0, d_head, 2, dtype=np.float32) / d_head))
    t = np.arange(seq_len, dtype=np.float32)
    freqs = np.einsum("i,j->ij", t, inv_freq)
    emb = np.concatenate([freqs, freqs], axis=-1)  # [T, dh]
    cosT = np.ascontiguousarray(np.cos(emb).astype(np.float32).T)  # [dh, T]
    sinT = np.ascontiguousarray(np.sin(emb).astype(np.float32).T)
    sgn = np.ones((d_head, 1), np.float32)
    sgn[: d_head // 2] = -1.0
    return cosT, sinT * sgn


def _causal_mask_add(tk):
    # Additive causal triangle [tk, tk]: with column-restricted diagonal
    # tiles the only region that ever needs masking is the tk x tk block
    # on the diagonal itself: entry is -1e30 (masked) iff c < r.
    m = np.zeros((tk, tk), np.float32)
    for r in range(tk):
        m[r, :r] = -1e30
    return m


def _legalize_waits(nc, mybir):
    """Walrus on this toolchain refuses more than one embedded sync wait
    per engine instruction. Hoist extra waits into standalone
    EventSemaphore instructions on the same engine queue (the sequencer
    executes them in-stream before the instruction, same gating)."""
    n = 0
    for f in nc.m.functions:
        for bb in f.blocks:
            out = []
            for inst in bb.instructions:
                si = inst.sync_info
                if (si and si.on_wait and len(si.on_wait) > 1
                        and not isinstance(inst, mybir.InstEventSemaphore)):
                    for w in si.on_wait[:-1]:
                        out.append(mybir.InstEventSemaphore(
                            name=f"WH-{n}", engine=inst.engine,
                            sync_info=mybir.SyncInfo(
                                on_wait=[w], on_update=[])))
                        n += 1
                    inst.sync_info = mybir.SyncInfo(
                        on_wait=[si.on_wait[-1]],
                        on_update=list(si.on_update))
                out.append(inst)
            bb.instructions = out
    return n


def _build_nc(b_sz, t_sz, d_sz, legalize=True):
    import concourse.bass as bass
    import concourse.tile as tile
    from concourse import mybir

    f32 = mybir.dt.float32
    f32r = mybir.dt.float32r
    bf16 = mybir.dt.bfloat16
    EXP = mybir.ActivationFunctionType.Exp
    LN = mybir.ActivationFunctionType.Ln

    DC = d_sz // 128         # contraction chunks
    DCH = DC // 2            # chunks per x half-tile
    NQG = t_sz // TQ         # q groups per (batch, head)
    NKT = t_sz // TK         # kv tiles
    KPG = TQ // TK           # kv tiles per q group (diagonal span)

    nc = bass.Bass("TRN2", target_bir_lowering=False, debug=False,
                   enable_asserts=False, dynamic_dma_scratch_size=2048)

    xT = nc.dram_tensor("xT", [b_sz, d_sz, t_sz], f32, kind="ExternalInput")
    wq = nc.dram_tensor("wq", [d_sz, HPC * DH], f32, kind="ExternalInput")
    wk = nc.dram_tensor("wk", [d_sz, HPC * DH], f32, kind="ExternalInput")
    wv = nc.dram_tensor("wv", [d_sz, HPC * DH], f32, kind="ExternalInput")
    wo = nc.dram_tensor("wo", [HPC * DH, d_sz], f32, kind="ExternalInput")
    cos = nc.dram_tensor("cos", [DH, t_sz], bf16, kind="ExternalInput")
    sin = nc.dram_tensor("sin", [DH, t_sz], bf16, kind="ExternalInput")
    msk = nc.dram_tensor("msk", [TK, TK], f32, kind="ExternalInput")
    idn = nc.dram_tensor("idn", [128, 128], f32, kind="ExternalInput")
    one = nc.dram_tensor("one", [128, 128], f32, kind="ExternalInput")
    y = nc.dram_tensor("y", [b_sz, t_sz, d_sz], bf16, kind="ExternalOutput")

    xT_r = xT.ap().rearrange("b (dc p) t -> b p dc t", p=128)
    wq_r = wq.ap().rearrange("(dc p) n -> p dc n", p=128)
    wk_r = wk.ap().rearrange("(dc p) n -> p dc n", p=128)
    wv_r = wv.ap().rearrange("(dc p) n -> p dc n", p=128)
    wo_r = wo.ap().rearrange("(h p) n -> p h n", p=128)
    y_r = y.ap()

    with tile.TileContext(nc) as tc:
        with (
            tc.tile_pool(name="consts", bufs=1) as consts,
            tc.tile_pool(name="wpool", bufs=1) as wpool,
            tc.tile_pool(name="qkv", bufs=1) as qkv,
            tc.tile_pool(name="xpool", bufs=3) as xpool,
            tc.tile_pool(name="rope", bufs=2) as rope,
            tc.tile_pool(name="pex", bufs=5) as pexp,
            tc.tile_pool(name="gpq", bufs=2) as gpq,
            tc.tile_pool(name="nrm", bufs=2) as nrmp,
            tc.tile_pool(name="otn", bufs=6) as otnp,
            tc.tile_pool(name="ysb", bufs=8) as ysbp,
            tc.tile_pool(name="psS", bufs=2, space="PSUM") as psS,
            tc.tile_pool(name="psO", bufs=2, space="PSUM") as psO,
            tc.tile_pool(name="psR", bufs=1, space="PSUM") as psR,
            tc.tile_pool(name="psY", bufs=2, space="PSUM") as psY,
        ):
            cos_sb = consts.tile([DH, t_sz], bf16)
            sin_sb = consts.tile([DH, t_sz], bf16)
            msk_sb = consts.tile([TK, TK], f32r)
            idn_sb = consts.tile([128, 128], f32r)
            # single [128,128] ones tile: column 0 is the sum-matmul
            # stationary; row 32*h is head h's reciprocal-broadcast
            # stationary (at base partition 32*h so the K=1 broadcast
            # matmuls of the two heads land in different PE row groups).
            onesq_sb = consts.tile([128, 128], f32r)

            wq_sb = wpool.tile([128, DC, HPC * DH], f32r)
            wk_sb = wpool.tile([128, DC, HPC * DH], f32r)
            wv_sb = wpool.tile([128, DC, HPC * DH], f32r)
            wo_sb = wpool.tile([128, HPC, d_sz], f32r)

            def load_x_half(xh, b, half, tsl):
                for dc in range(DCH):
                    nc.sync.dma_start(
                        xh[:, dc, :],
                        xT_r[b, :, half * DCH + dc, tsl].bitcast(f32r))

            # first-needed data first: the first x half-tile and q weight
            # chunks feed the very first matmuls, so their DMAs go at the
            # head of every queue; wk/wv/cos/sin follow in consumption
            # order.
            xt_first = [xpool.tile([128, DCH, TT], f32r, tag="xt",
                                   name="xt_first") for _ in range(2)]
            for half in range(2):
                for dc in range(DCH):
                    nc.sync.dma_start(
                        xt_first[half][:, dc, :],
                        xT_r[0, :, half * DCH + dc, 0:TT].bitcast(f32r))
                    nc.sync.dma_start(
                        wq_sb[:, half * DCH + dc, :],
                        wq_r[:, half * DCH + dc, :].bitcast(f32r))
            nc.sync.dma_start(cos_sb[:, 0:TT], cos.ap()[:, 0:TT])
            nc.sync.dma_start(sin_sb[:, 0:TT], sin.ap()[:, 0:TT])
            for dc in range(DC):
                nc.sync.dma_start(wk_sb[:, dc, :],
                                  wk_r[:, dc, :].bitcast(f32r))
            for dc in range(DC):
                nc.sync.dma_start(wv_sb[:, dc, :],
                                  wv_r[:, dc, :].bitcast(f32r))
            # batch 0 is DMA-bound: prefetch the lo half of its second x
            # tile right after the weights (only one ring slot is free -
            # prefetching the hi half would head-of-line block the consts
            # behind it in its DMA queue)
            xt_b0t1 = xpool.tile([128, DCH, TT], f32r, tag="xt",
                                 name="xt_b0t1")
            load_x_half(xt_b0t1, 0, 0, slice(TT, 2 * TT))

            def load_consts():
                # emitted after the first x tile's DMAs: nothing here is
                # needed before attention of the first tile
                for i in range(1, t_sz // TT):
                    sl = slice(i * TT, (i + 1) * TT)
                    nc.sync.dma_start(cos_sb[:, sl], cos.ap()[:, sl])
                    nc.sync.dma_start(sin_sb[:, sl], sin.ap()[:, sl])
                nc.sync.dma_start(msk_sb[:], msk.ap().bitcast(f32r))
                nc.sync.dma_start(idn_sb[:], idn.ap().bitcast(f32r))
                nc.sync.dma_start(onesq_sb[:], one.ap().bitcast(f32r))

            def load_wo():
                # deferred past all of batch 0's x tiles so the 8.4MB of wo
                # doesn't sit ahead of them in the DMA queue FIFOs; first
                # needed by the first out-projection, ~25us into phase B
                for hh in range(HPC):
                    for nch in range(d_sz // 512):
                        nsl = slice(nch * 512, (nch + 1) * 512)
                        nc.sync.dma_start(wo_sb[:, hh, nsl],
                                          wo_r[:, hh, nsl].bitcast(f32r))

            pending = []
            norm_pending = []
            otn_tiles = {}
            xt_prefetch = {(0, 1, 0): xt_b0t1}

            for b in range(b_sz):
                # ------- merged phases: proj tile tt, then attention for
                # q-group qi=tt (needs only qT/kT/vv from tiles <= tt).
                # Attention work overlaps the x/weight DMA of later tiles,
                # which hides batch 0's DMA-bound start and removes the
                # phase/batch boundary stalls.
                kT = [qkv.tile([DH, t_sz], f32r, tag=f"kT{h}", name=f"kT{h}")
                      for h in range(HPC)]
                vv = qkv.tile([128, NKT, HPC * DH], f32r, tag="vv",
                              name="vv")

                for tt in range(t_sz // TT):
                    tsl = slice(tt * TT, (tt + 1) * TT)
                    if b == 0 and tt == 0:
                        xt = xt_first
                        load_consts()
                    else:
                        xt = []
                        for half in range(2):
                            if (b, tt, half) in xt_prefetch:
                                xt.append(xt_prefetch.pop((b, tt, half)))
                            else:
                                xh = xpool.tile([128, DCH, TT], f32r,
                                                tag="xt", name="xt")
                                load_x_half(xh, b, half, tsl)
                                xt.append(xh)

                    # q is only read by this tile's own q-group (qi == tt),
                    # so it lives in a small per-tile ring instead of a
                    # full [DH, T] buffer (frees SBUF for the x prefetch)
                    qTt = [qkv.tile([DH, TT], f32r, tag=f"qT{h}", bufs=2,
                                    name=f"qT{h}") for h in range(HPC)]
                    for h in range(HPC):
                        hs = slice(h * DH, (h + 1) * DH)
                        for dst, w_sb in ((qTt[h][:, :], wq_sb),
                                          (kT[h][:, tsl], wk_sb)):
                            pp = psS.tile([128, TT], f32, tag="st", name="pp")
                            for dc in range(DC):
                                nc.tensor.matmul(
                                    pp[:],
                                    w_sb[:, dc, hs],
                                    xt[dc // DCH][:, dc % DCH, :],
                                    start=(dc == 0), stop=(dc == DC - 1),
                                )
                            # RoPE: dst = ppc*cos + swap(ppc)*sin_signed.
                            # The pp PSUM bank is freed by a fast ACT copy
                            # (if the DVE reads pp directly, the st-ring
                            # couples the PE to DVE backlog). The
                            # rotate-half partition swap runs on the DMA
                            # engine (the DVE cannot pair SBUF operands at
                            # different start partitions).
                            ppc = rope.tile([128, TT], bf16, tag="ppc",
                                            name="ppc")
                            nc.scalar.copy(ppc[:], pp[:])
                            psw = rope.tile([128, TT], bf16, tag="psw",
                                            name="psw")
                            nc.sync.dma_start(psw[0:64, :], ppc[64:128, :])
                            nc.sync.dma_start(psw[64:128, :], ppc[0:64, :])
                            sh = rope.tile([128, TT], bf16, tag="sh",
                                           bufs=2, name="sh")
                            nc.vector.tensor_mul(sh[:], psw[:],
                                                 sin_sb[:, tsl])
                            nc.vector.tensor_mul(dst, ppc[:],
                                                 cos_sb[:, tsl])
                            nc.vector.tensor_add(dst, dst, sh[:])

                    for ts2 in range(TT // TK):
                        vp = psS.tile([TK, HPC * DH], f32, tag="st",
                                      name="vp")
                        for dc in range(DC):
                            nc.tensor.matmul(
                                vp[:],
                                xt[dc // DCH][:, dc % DCH,
                                              ts2 * TK:(ts2 + 1) * TK],
                                wv_sb[:, dc, :],
                                start=(dc == 0), stop=(dc == DC - 1),
                            )
                        kv_i = tt * (TT // TK) + ts2
                        nc.scalar.copy(vv[:, kv_i, :], vp[:])

                    # prefetch one x half-tile ahead (exactly one ring
                    # slot is free), so its DMA sits ahead of the y-output
                    # traffic of the attention below
                    nt_b, nt_tt = (b, tt + 1) if tt + 1 < t_sz // TT \
                        else (b + 1, 0)
                    if nt_b < b_sz and (nt_b, nt_tt) != (0, 1):
                        xh = xpool.tile([128, DCH, TT], f32r, tag="xt",
                                        name="xt")
                        load_x_half(xh, nt_b, 0,
                                    slice(nt_tt * TT, (nt_tt + 1) * TT))
                        xt_prefetch[(nt_b, nt_tt, 0)] = xh

                    # ---------------- attention + out proj for qi=tt ----
                    # both heads interleave within the q-group: every
                    # cross-engine chain (exp -> PV) gets two kv-tiles of
                    # PE work as cover. The causal mask is an identity-
                    # matmul accumulation of an additive -1e30 master
                    # slice into the score PSUM (PE-side: no cross-engine
                    # latency on the exp input).
                    qi = tt
                    nkv = KPG * (qi + 1)
                    outp = [psO.tile([DH, TQ], f32, tag="outT",
                                     name=f"outp{h}") for h in range(HPC)]
                    sump = [psR.tile([1, TQ], f32, tag=f"sums{h}",
                                     name=f"sump{h}") for h in range(HPC)]
                    # off-diagonal exp tiles are pre-summed in quads of 4
                    # on the otherwise-idle GpSimd engine; only one ones-
                    # matmul per quad reaches the PE (flushed at group
                    # end, where the 4 trailing diagonal tiles give the
                    # GpSimd add chain cover). Each group's off-diagonal
                    # count 4*qi is an exact multiple of 4.
                    stash = [None] * HPC
                    qac = [None] * HPC
                    quad_flush = []
                    prev = None
                    for ki in range(nkv):
                        # column-restrict diagonal tiles: kv tile at
                        # diagonal offset dg only attends queries >= dg*TK
                        # within the group; everything below is fully
                        # masked, so score/exp/PV/sum skip those columns
                        # and the mask shrinks to the TK x TK triangle.
                        dg = ki - KPG * qi
                        w0 = 0 if dg < 0 else dg * TK
                        pex = []
                        for h in range(HPC):
                            stp = psS.tile([TK, TQ], f32, tag="st",
                                           name="stp")
                            nc.tensor.matmul(
                                stp[:, w0:],
                                kT[h][:, ki * TK:(ki + 1) * TK],
                                qTt[h][:, w0:],
                                start=True, stop=(dg < 0),
                            )
                            if dg >= 0:
                                nc.tensor.matmul(
                                    stp[:, w0:w0 + TK], idn_sb[:],
                                    msk_sb[:],
                                    start=False, stop=True,
                                )
                            px = pexp.tile([TK, TQ], f32r, tag="pex",
                                           name="pex")
                            nc.scalar.activation(px[:, w0:], stp[:, w0:],
                                                 EXP)
                            pex.append(px)
                            # off-diagonal quad pre-sum on GpSimd (runs in
                            # parallel with the PE; emitted here so the
                            # pex ring slot's reader exists before the
                            # slot is recycled two iterations later)
                            if dg < 0:
                                if ki % 4 == 0:
                                    stash[h] = px
                                elif ki % 4 == 1:
                                    qac[h] = gpq.tile([128, TQ], f32r,
                                                      tag=f"qac{h}",
                                                      bufs=3,
                                                      name=f"qac{h}")
                                    nc.gpsimd.tensor_add(
                                        qac[h][:], stash[h][:], px[:])
                                else:
                                    nc.gpsimd.tensor_add(
                                        qac[h][:], qac[h][:], px[:])
                                    if ki % 4 == 3:
                                        quad_flush.append((h, qac[h]))
                        # PV+sum of the PREVIOUS kv tile go to the PE now,
                        # so the exps hide under the next scores
                        if prev is not None:
                            pki, pw0, ppex = prev
                            pdg = pki - KPG * qi
                            for h in range(HPC):
                                nc.tensor.matmul(
                                    outp[h][:, pw0:],
                                    vv[:, pki, h * DH:(h + 1) * DH],
                                    ppex[h][:, pw0:], start=(pki == 0),
                                    stop=False,
                                )
                            if pdg >= 0:
                                for h in range(HPC):
                                    nc.tensor.matmul(
                                        sump[h][:, pw0:], onesq_sb[:, 0:1],
                                        ppex[h][:, pw0:],
                                        start=(pdg == 0), stop=False,
                                    )
                        prev = (ki, w0, pex)
                        # out-proj chunk first (independent PE work whose
                        # psY slot is long freed), THEN the norm chain
                        # whose rbc matmul depends on fresh ACT/DVE work.
                        # Norm chains pop at ki 2..3 (not 0..1): their
                        # [1,TQ] LN/EXP table ops then never delay the new
                        # group's first exps, which gate the score-psum
                        # ring reuse at the boundary
                        if len(pending) > 4:
                            pending.pop(0)()
                        if 2 <= ki < 2 + HPC and norm_pending:
                            norm_pending.pop(0)()
                    pki, pw0, ppex = prev
                    for h in range(HPC):
                        nc.tensor.matmul(
                            outp[h][:, pw0:],
                            vv[:, pki, h * DH:(h + 1) * DH],
                            ppex[h][:, pw0:], start=(pki == 0), stop=True,
                        )
                    for h in range(HPC):
                        nc.tensor.matmul(
                            sump[h][:, pw0:], onesq_sb[:, 0:1],
                            ppex[h][:, pw0:],
                            start=(pki - KPG * qi == 0), stop=(qi == 0),
                        )
                    for i, (h, qa) in enumerate(quad_flush):
                        nc.tensor.matmul(
                            sump[h][:], onesq_sb[:, 0:1], qa[:],
                            start=False,
                            stop=(i >= len(quad_flush) - HPC),
                        )

                    # copy the unnormalized attention output to SBUF right
                    # away: this frees the psO bank after one ACT op, so
                    # the next group's first PV matmul never waits for the
                    # (deferred, ~3.5us) normalization chain. The chain
                    # then scales otn in place.
                    # (on DVE, not ACT: an ACT copy here sits in the
                    # in-order ACT queue between this group's exps and the
                    # next group's, delaying the exp that frees the score
                    # psum ring)
                    otn_now = []
                    for h in range(HPC):
                        otn = otnp.tile([DH, TQ], f32r, tag="otn",
                                        name="otn")
                        nc.vector.tensor_copy(otn[:], outp[h][:])
                        otn_now.append(otn)

                    # normalization chains, deferred one kv-tile into the
                    # NEXT q-group so their ACT ops (LN, exp(-x)) never
                    # sit between consecutive exps at a group boundary.
                    # rcp = exp(-ln(den)) on ACT (same table set as
                    # Exp/Copy: no ACT_TABLE_LOAD); a ones-matmul
                    # broadcasts it across partitions into a psY-ring
                    # bank.
                    for h in range(HPC):
                        def norm_group(h=h, qi=qi, b=b, otn=otn_now[h],
                                       sump=sump[h]):
                            lnd = nrmp.tile([1, TQ], f32, tag="lnd",
                                            bufs=1, name="lnd")
                            nc.scalar.activation(lnd[:], sump[:], LN)
                            rcp1 = nrmp.tile([1, TQ], f32r, tag="rcp1",
                                             bufs=1, name="rcp1")
                            nc.scalar.activation(rcp1[:], lnd[:], EXP,
                                                 scale=-1.0)
                            rbc = psY.tile([DH, TQ], f32, tag="y",
                                           name="rbc")
                            nc.tensor.matmul(rbc[:],
                                             onesq_sb[0:1, :],
                                             rcp1[:], start=True,
                                             stop=True)
                            rbs = nrmp.tile([DH, TQ], bf16, tag="rbs",
                                            name="rbs")
                            nc.vector.tensor_copy(rbs[:], rbc[:])
                            nc.vector.tensor_mul(otn[:], otn[:], rbs[:])
                            otn_tiles[(h, qi)] = otn

                        norm_pending.append(norm_group)

                    def out_chunk(qi=qi, b=b):
                        def chunk(tc2, qi=qi, b=b):
                            tq0 = qi * TQ + tc2 * TK
                            for nch in range(d_sz // 512):
                                yp = psY.tile([TK, 512], f32,
                                              tag="y", name="yp")
                                for hh in range(HPC):
                                    nc.tensor.matmul(
                                        yp[:],
                                        otn_tiles[(hh, qi)][
                                            :, tc2 * TK:(tc2 + 1) * TK],
                                        wo_sb[:, hh,
                                              nch * 512:(nch + 1) * 512],
                                        start=(hh == 0),
                                        stop=(hh == HPC - 1),
                                    )
                                ysb = ysbp.tile([TK, 512], bf16,
                                                tag="ysb", name="ysb")
                                nc.vector.tensor_copy(ysb[:], yp[:])
                                nc.sync.dma_start(
                                    y_r[b, tq0:tq0 + TK,
                                        nch * 512:(nch + 1) * 512],
                                    ysb[:])
                        return [lambda tc2=tc2: chunk(tc2)
                                for tc2 in range(TQ // TK)]

                    pending.extend(out_chunk())

                    if b == 0 and tt == 1:
                        load_wo()

            for fn in norm_pending:
                fn()
            for fn in pending:
                fn()
    if legalize:
        _legalize_waits(nc, mybir)
    return nc


_NC_CACHE = {}
LAST_RESULT = None


def _get_nc(b_sz, t_sz, d_sz):
    key = (b_sz, t_sz, d_sz)
    if key not in _NC_CACHE:
        _NC_CACHE[key] = _build_nc(b_sz, t_sz, d_sz)
    return _NC_CACHE[key]


def kernel(x, w_q, w_k, w_v, w_o):
    import ml_dtypes
    from concourse.bass_utils import run_bass_kernel_spmd

    BF = ml_dtypes.bfloat16
    b_sz, t_sz, d_sz = x.shape
    scale = np.float32(1.0 / np.sqrt(DH))

    xT = np.ascontiguousarray(np.asarray(x, np.float32).transpose(0, 2, 1))
    w_q = np.asarray(w_q, np.float32)
    w_k = np.asarray(w_k, np.float32)
    w_v = np.asarray(w_v, np.float32)
    w_o = np.asarray(w_o, np.float32)
    cosT, sinT = _rope_tables(t_sz, DH, THETA)
    mask = _causal_mask_add(TK)

    in_maps = []
    for c in range(NCORES):
        cs = slice(c * HPC * DH, (c + 1) * HPC * DH)
        in_maps.append({
            "xT": xT,
            "wq": np.ascontiguousarray(w_q[:, cs] * scale),
            "wk": np.ascontiguousarray(w_k[:, cs]),
            "wv": np.ascontiguousarray(w_v[:, cs]),
            "wo": np.ascontiguousarray(w_o[cs, :]),
            "cos": cosT.astype(BF),
            "sin": sinT.astype(BF),
            "msk": mask,
            "idn": np.eye(128, dtype=np.float32),
            "one": np.ones((128, 128), np.float32),
        })

    nc = _get_nc(b_sz, t_sz, d_sz)
    res = run_bass_kernel_spmd(nc, in_maps, core_ids=list(range(NCORES)))
    global LAST_RESULT
    LAST_RESULT = res

    out = np.asarray(res.results[0]["y"]).astype(np.float32)
    for c in range(1, NCORES):
        out += np.asarray(res.results[c]["y"]).astype(np.float32)
    return out

